# revision 1
# baseline (speedup 1.0000x reference)
"""Trainium2 Bass kernel for ContextAwareMissingEmbeddingGenerator.

Data-parallel over batch: 8 cores x 512 samples. The module is algebraically
restructured so the only heavy device work is one fused projection
y = G @ x^T per row-block, where G = [U(184); Mcat/S(400); Wp/S(50)]:
  - U[(h,q),:]  = Wk_h^T qm[q,h] / sqrt(HD)    (scores vs constant missing-queries)
  - Mcat[(h,l)] = (Wp Wo)_h Wv_h / S           (attention value path folded to L dims)
  - Wp/S                                        (document-mean path)
Attention softmax runs over the free dim in [head*query, row] layout; the
query-sum, head-broadcast and head-sum contractions are one-hot matmuls.
"""

import math
from contextlib import ExitStack

import numpy as np

import concourse.bass as bass
import concourse.bacc as bacc_mod
import concourse.mybir as mybir
import concourse.tile as tile
from concourse.bass_utils import run_bass_kernel_spmd

D, H, HD, S, L, B = 768, 8, 96, 23, 50, 4096
NCORES = 8
BC = B // NCORES              # samples per core
ROWS = BC * S                 # 11776 rows per core
NBF = 22                      # samples per full block
NBLK_F = BC // NBF            # 23 full blocks
NB_TAIL = BC - NBF * NBLK_F   # 6
GN = 634                      # G rows
YCH = [(0, 128), (128, 256), (256, 384), (384, 512), (512, 634)]
NEG = -30000.0

F32 = mybir.dt.float32
F32R = mybir.dt.float32r

MM_DT = F32R                  # dtype for the bulk matmuls


def _host_prep(cls_emb, missing_table, in_proj_w, in_proj_b,
               out_proj_w, out_proj_b, pred_w, pred_b, exist_mask):
    f32 = np.float32
    x = np.ascontiguousarray(cls_emb, dtype=f32)
    mt = np.asarray(missing_table, f32)
    ipw = np.asarray(in_proj_w, f32)
    ipb = np.asarray(in_proj_b, f32)
    opw = np.asarray(out_proj_w, f32)
    opb = np.asarray(out_proj_b, f32)
    pw = np.asarray(pred_w, f32)
    pb = np.asarray(pred_b, f32)
    em = np.asarray(exist_mask)

    Wq, Wk, Wv = ipw[0:D], ipw[D:2 * D], ipw[2 * D:3 * D]
    bq, bk, bv = ipb[0:D], ipb[D:2 * D], ipb[2 * D:3 * D]
    scale = 1.0 / math.sqrt(HD)
    qm = mt @ Wq.T + bq
    qh = qm.reshape(S, H, HD)
    Wk3 = Wk.reshape(H, HD, D)
    Wv3 = Wv.reshape(H, HD, D)
    U = (np.einsum('hij,qhi->hqj', Wk3, qh) * scale).reshape(H * S, D)
    c0 = (np.einsum('qhi,hi->hq', qh, bk.reshape(H, HD)) * scale).reshape(H * S)
    W2 = pw @ opw
    Mcat = np.einsum('lhi,hid->hld', W2.reshape(L, H, HD), Wv3).reshape(H * L, D) / S
    G = np.concatenate([U, Mcat, pw / S], axis=0)          # [634, 768]
    Gt = np.ascontiguousarray(G.T, dtype=f32)              # [768, 634]
    vbs = ((W2 @ bv + pw @ opb) / S).astype(f32)
    wpts = ((pw @ mt.sum(0)) / S).astype(f32)

    m = em.astype(f32)                                     # [B, S]
    hasany = (m.sum(1) > 0).astype(f32)
    u = (1.0 - m) * hasany[:, None]
    nupd = u.sum(1)

    # one-hot / constant matmul operands
    obdp = np.zeros((H * S, 512), f32)     # A_rep producer, cols by y-chunk 1..4
    fp = np.zeros((512, L), f32)           # head-sum reducer
    for c in (1, 2, 3, 4):
        lo, hi = YCH[c]
        for p in range(hi - lo):
            g = lo + p
            col = (c - 1) * 128 + p
            if 184 <= g < 584:
                arow = g - 184
                j, l = arow // L, arow % L
                obdp[j * S:(j + 1) * S, col] = 1.0
                fp[col, l] = 1.0
            elif 584 <= g < 634:
                fp[col, (g - 584) % L] = 1.0
    k1pp = np.zeros((1, 122), f32)
    k1pp[0, 72:122] = 1.0                  # Wp rows inside y-chunk 4
    k1 = np.zeros((1, 406), f32)
    k1[0, 0:184] = NEG          # [0:128] mb chunk0; [128:184] mb chunk1, rest 0-pad
    k1[0, 256:306] = vbs
    k1[0, 306:356] = wpts
    k1[0, 356:406] = pb

    c0col = np.ascontiguousarray(c0.reshape(H * S, 1), f32)
    ident = np.eye(128, dtype=f32)

    # per-core shards
    shards = []
    for c in range(NCORES):
        b0, b1 = c * BC, (c + 1) * BC
        ms = m[b0:b1]
        urep = np.zeros((H * S, BC), f32)
        for h in range(H):
            urep[h * S:(h + 1) * S, :] = u[b0:b1].T
        svec = np.stack([nupd[b0:b1], 1.0 - hasany[b0:b1],
                         np.ones(BC, f32)]).astype(f32)
        mrow = np.stack([ms.reshape(-1), 1.0 - ms.reshape(-1)]).astype(f32)
        shards.append({
            "x": np.ascontiguousarray(x[b0:b1].reshape(ROWS, D)),
            "mrow": np.ascontiguousarray(mrow),
            "urep": np.ascontiguousarray(urep),
            "svec": np.ascontiguousarray(svec),
            "gt": Gt, "obdp": obdp, "fp": np.ascontiguousarray(fp),
            "k1pp": k1pp, "k1": k1, "c0col": c0col, "ident": ident,
        })
    return shards


def _mm(nc, out, lhsT, rhs, start, stop, dt=None):
    if dt is not None:
        lhsT = lhsT.bitcast(dt)
        rhs = rhs.bitcast(dt)
    nc.tensor.matmul(out, lhsT, rhs, start=start, stop=stop)


def _build_program():
    nc = bacc_mod.Bacc("TRN2", target_bir_lowering=False, debug=False)
    dt = F32
    x_d = nc.dram_tensor("x", [ROWS, D], dt, kind="ExternalInput").ap()
    mrow_d = nc.dram_tensor("mrow", [2, ROWS], F32R, kind="ExternalInput").ap()
    urep_d = nc.dram_tensor("urep", [H * S, BC], dt, kind="ExternalInput").ap()
    svec_d = nc.dram_tensor("svec", [3, BC], dt, kind="ExternalInput").ap()
    gt_d = nc.dram_tensor("gt", [D, GN], F32R, kind="ExternalInput").ap()
    obdp_d = nc.dram_tensor("obdp", [H * S, 512], F32R, kind="ExternalInput").ap()
    fp_d = nc.dram_tensor("fp", [512, L], dt, kind="ExternalInput").ap()
    k1pp_d = nc.dram_tensor("k1pp", [1, 122], F32R, kind="ExternalInput").ap()
    k1_d = nc.dram_tensor("k1", [1, 406], F32R, kind="ExternalInput").ap()
    c0_d = nc.dram_tensor("c0col", [H * S, 1], dt, kind="ExternalInput").ap()
    id_d = nc.dram_tensor("ident", [128, 128], dt, kind="ExternalInput").ap()
    out_d = nc.dram_tensor("logitsT", [L, BC], dt, kind="ExternalOutput").ap()

    with tile.TileContext(nc) as tc, ExitStack() as ctx:
        cpool = ctx.enter_context(tc.tile_pool(name="consts", bufs=1))
        natp = ctx.enter_context(tc.tile_pool(name="xnat", bufs=3))
        xtp = ctx.enter_context(tc.tile_pool(name="xt", bufs=2))
        ewp = ctx.enter_context(tc.tile_pool(name="ew", bufs=2))
        smp = ctx.enter_context(tc.tile_pool(name="small", bufs=2))
        scrp = ctx.enter_context(tc.tile_pool(name="scr", bufs=2))
        outp = ctx.enter_context(tc.tile_pool(name="outp", bufs=1))
        yp = ctx.enter_context(tc.tile_pool(name="ypsum", bufs=1, space="PSUM"))
        tp = ctx.enter_context(tc.tile_pool(name="tpsum", bufs=2, space="PSUM"))
        app = ctx.enter_context(tc.tile_pool(name="apsum", bufs=1, space="PSUM"))

        def cload(name, shape, src, cdt=F32):
            t = cpool.tile(shape, cdt, tag=name, name=name)
            nc.sync.dma_start(t[:], src)
            return t

        gt_sb = [cload(f"gt{dc}", [128, GN], gt_d[dc * 128:(dc + 1) * 128, :], F32R)
                 for dc in range(6)]
        ob0 = cload("ob0", [128, 512], obdp_d[0:128, :], F32R)
        ob1 = cload("ob1", [56, 512], obdp_d[128:184, :], F32R)
        fp_sb = [cload(f"fp{c}", [YCH[c][1] - YCH[c][0], L],
                       fp_d[(c - 1) * 128:(c - 1) * 128 + YCH[c][1] - YCH[c][0], :])
                 for c in (1, 2, 3, 4)]
        k1pp_sb = cload("k1pp", [1, 122], k1pp_d[:, :], F32R)
        k1_sb = cload("k1", [1, 406], k1_d[:, :], F32R)
        c0_sb0 = cload("c00", [128, 1], c0_d[0:128, :])
        c0_sb1 = cload("c01", [56, 1], c0_d[128:184, :])
        ur0 = cload("ur0", [128, BC], urep_d[0:128, :])
        ur1 = cload("ur1", [56, BC], urep_d[128:184, :])
        sv0 = cload("sv0", [1, BC], svec_d[0:1, :])
        sv1 = cload("sv1", [1, BC], svec_d[1:2, :])
        sv2 = cload("sv2", [1, BC], svec_d[2:3, :])
        id_sb = cload("id", [128, 128], id_d[:, :])
        outT = outp.tile([L, BC], F32, tag="outT", name="outT")

        for blk in range(NBLK_F + (1 if NB_TAIL else 0)):
            nb = NBF if blk < NBLK_F else NB_TAIL
            N = nb * S
            r0 = blk * NBF * S
            b0 = blk * NBF
            groups = []
            off = 0
            while off < N:
                pg = min(128, N - off)
                groups.append((off, pg))
                off += pg

            mrp = smp.tile([1, N], F32R, tag="mrp", name=f"mrp{blk}")
            nc.sync.dma_start(mrp[:], mrow_d[0:1, r0:r0 + N])
            mrn = smp.tile([1, N], F32R, tag="mrn", name=f"mrn{blk}")
            nc.sync.dma_start(mrn[:], mrow_d[1:2, r0:r0 + N])

            # ---- load + transpose x ----
            xts = [xtp.tile([128, N], F32R, tag=f"xt{dc}", name=f"xt{dc}_{blk}") for dc in range(6)]
            for gi, (goff, pg) in enumerate(groups):
                xn = natp.tile([pg, D], F32, tag="xn", name=f"xn{blk}_{gi}")
                nc.sync.dma_start(xn[:], x_d[r0 + goff:r0 + goff + pg, :])
                for dc in range(6):
                    tt = tp.tile([128, pg], F32, tag="tp", name=f"tt{blk}_{gi}_{dc}")
                    nc.tensor.transpose(tt[:], xn[:, dc * 128:(dc + 1) * 128],
                                        id_sb[:pg, :pg])
                    dst = xts[dc][:, goff:goff + pg]
                    if dc % 2 == 0:
                        nc.vector.tensor_copy(dst, tt[:])
                    else:
                        nc.scalar.copy(dst, tt[:])

            # ---- G matmuls: chunks 0,1 (scores) + mask bias ----
            ys = [None] * 5
            for c in (0, 1):
                lo, hi = YCH[c]
                yt = yp.tile([hi - lo, N], F32, tag=f"y{c}", name=f"y{c}_{blk}")
                ys[c] = yt
                for dc in range(6):
                    _mm(nc, yt[:], gt_sb[dc][:, lo:hi], xts[dc][:],
                        start=(dc == 0), stop=False, dt=MM_DT)
                if c == 0:
                    _mm(nc, yt[:], k1_sb[:, 0:128], mrn[:],
                        start=False, stop=True, dt=MM_DT)
                else:
                    _mm(nc, yt[:], k1_sb[:, 128:256], mrn[:],
                        start=False, stop=True, dt=MM_DT)

            # ---- proj chunk copies to SBUF (DVE/ACT 2-PSUM-operand limit) ----
            ysb = [None] * 5
            yt1 = ysb[1] = scrp.tile([128, N], F32, tag="ysb1", name=f"ysb1_{blk}")
            nc.scalar.copy(yt1[:], ys[1][:])

            # ---- exp ----
            e0 = ewp.tile([128, N], F32, tag="e0", name=f"e0_{blk}")
            e1 = ewp.tile([56, N], F32, tag="e1", name=f"e1_{blk}")
            nc.scalar.activation(e0[:], ys[0][:],
                                 mybir.ActivationFunctionType.Exp, bias=c0_sb0[:])
            nc.scalar.activation(e1[:], ys[1][0:56, :],
                                 mybir.ActivationFunctionType.Exp, bias=c0_sb1[:])

            # ---- G matmuls: chunks 2..4 ----
            for c in (2, 3, 4):
                lo, hi = YCH[c]
                yt = yp.tile([hi - lo, N], F32, tag=f"y{c}", name=f"y{c}_{blk}")
                ys[c] = yt
                for dc in range(6):
                    _mm(nc, yt[:], gt_sb[dc][:, lo:hi], xts[dc][:],
                        start=(dc == 0), stop=(dc == 5), dt=MM_DT)
                ysb[c] = scrp.tile([hi - lo, N], F32, tag=f"ysb{c}",
                                   name=f"ysb{c}_{blk}")
                if c == 2:
                    nc.scalar.copy(ysb[c][:], yt[:])
                else:
                    nc.vector.tensor_copy(ysb[c][:], yt[:])

            # ---- softmax denom + query weights ----
            den0 = smp.tile([128, nb], F32, tag="den0", name=f"den0_{blk}")
            den1 = smp.tile([56, nb], F32, tag="den1", name=f"den1_{blk}")
            nc.vector.tensor_reduce(den0[:], e0[:].rearrange("p (b k) -> p b k", k=S),
                                    axis=mybir.AxisListType.X, op=mybir.AluOpType.add)
            nc.vector.tensor_reduce(den1[:], e1[:].rearrange("p (b k) -> p b k", k=S),
                                    axis=mybir.AxisListType.X, op=mybir.AluOpType.add)
            up0 = smp.tile([128, nb], F32, tag="up0", name=f"up0_{blk}")
            up1 = smp.tile([56, nb], F32, tag="up1", name=f"up1_{blk}")
            nc.vector.tensor_scalar_add(den0[:], den0[:], 1e-30)
            nc.vector.tensor_scalar_add(den1[:], den1[:], 1e-30)
            nc.vector.reciprocal(up0[:], den0[:])
            nc.vector.reciprocal(up1[:], den1[:])
            nc.vector.tensor_mul(up0[:], up0[:], ur0[:, b0:b0 + nb])
            nc.vector.tensor_mul(up1[:], up1[:], ur1[:, b0:b0 + nb])

            # ---- w = e * u' (broadcast over k) ----
            w0 = ewp.tile([128, N], F32R, tag="w0", name=f"w0_{blk}")
            w1 = ewp.tile([56, N], F32R, tag="w1", name=f"w1_{blk}")
            bc0 = up0[:].rearrange("p (b o) -> p b o", o=1).broadcast_to([128, nb, S])
            bc1 = up1[:].rearrange("p (b o) -> p b o", o=1).broadcast_to([56, nb, S])
            nc.vector.tensor_mul(w0[:].rearrange("p (b k) -> p b k", k=S),
                                 e0[:].rearrange("p (b k) -> p b k", k=S), bc0)
            nc.vector.tensor_mul(w1[:].rearrange("p (b k) -> p b k", k=S),
                                 e1[:].rearrange("p (b k) -> p b k", k=S), bc1)

            # ---- A_rep chunks + weighted reduce ----
            cps = []
            for c in (1, 2, 3, 4):
                lo, hi = YCH[c]
                pc = hi - lo
                ar = app.tile([pc, N], F32, tag="ap", name=f"ar{blk}_{c}")
                cols = slice((c - 1) * 128, (c - 1) * 128 + pc)
                _mm(nc, ar[:], ob0[:, cols], w0[:], start=True, stop=False, dt=MM_DT)
                _mm(nc, ar[:], ob1[:, cols], w1[:], start=False,
                    stop=(c != 4), dt=MM_DT)
                if c == 4:
                    _mm(nc, ar[:], k1pp_sb[:, :], mrp[:],
                        start=False, stop=True, dt=MM_DT)
                ps = scrp.tile([pc, N], F32, tag="ps", name=f"ps{blk}_{c}")
                nc.vector.tensor_mul(ps[:], ar[:], ysb[c][:])
                cp = smp.tile([pc, nb], F32, tag=f"cp{c}", name=f"cp{blk}_{c}")
                nc.vector.tensor_reduce(cp[:],
                                        ps[:].rearrange("p (b k) -> p b k", k=S),
                                        axis=mybir.AxisListType.X,
                                        op=mybir.AluOpType.add)
                cps.append((c, pc, cp))

            # ---- head-sum + rank-1 terms -> logits block ----
            ct = app.tile([L, nb], F32, tag="ap", name=f"ct{blk}")
            for i, (c, pc, cp) in enumerate(cps):
                _mm(nc, ct[:], fp_sb[i][:], cp[:], start=(i == 0), stop=False)
            _mm(nc, ct[:], k1_sb[:, 256:306].bitcast(F32), sv0[:, b0:b0 + nb],
                start=False, stop=False)
            _mm(nc, ct[:], k1_sb[:, 306:356].bitcast(F32), sv1[:, b0:b0 + nb],
                start=False, stop=False)
            _mm(nc, ct[:], k1_sb[:, 356:406].bitcast(F32), sv2[:, b0:b0 + nb],
                start=False, stop=True)
            nc.vector.tensor_copy(outT[:, b0:b0 + nb], ct[:])

        nc.sync.dma_start(out_d[:, :], outT[:])
    nc.compile()
    return nc


_CACHED = {}


def _get_program():
    if "nc" not in _CACHED:
        _CACHED["nc"] = _build_program()
    return _CACHED["nc"]


def _run(inputs, trace=False):
    shards = _host_prep(**inputs)
    nc = _get_program()
    res = run_bass_kernel_spmd(nc, shards, list(range(NCORES)), trace=trace)
    outs = [res.results[i]["logitsT"] for i in range(NCORES)]
    full = np.concatenate(outs, axis=1).T.astype(np.float32)
    return full, res


def kernel(**inputs):
    out, _ = _run(inputs, trace=False)
    return out


def run_traced(inputs):
    return _run(inputs, trace=True)



# revision 41
# speedup vs baseline: 1.6944x; 1.6944x over previous
"""Trainium2 Bass kernel for ContextAwareMissingEmbeddingGenerator.

Data-parallel over batch: 8 cores x 512 samples. Per-row work is split into
two matmul paths:
  - scores: y = U @ x^T (184 rows, bf16) where U folds Wk against the
    constant missing-table queries; x^T is produced by the XBAR DMA-transpose
    (no PE time). Softmax runs over the free dim in [head*query, row] layout.
  - values/doc-mean: instead of folding (pred@out_proj@Wv) into a 400-row
    per-row matmul, attention weights are reduced to per-(head,row) sums
    (A9T, one-hot matmuls), expanded into block one-hot weight tiles (Abm),
    and applied as xw[d,(h,b)] = sum_r x[r,d]*Abm[r,(h,b)] using x in its
    NATURAL layout as the stationary operand. A 9th "head" carries the
    masked doc-mean. 54 tiny matmuls against (W2_h Wv_h)/S and Wp/S produce
    logits directly.
Constant parts (value bias, missing-table means, pred bias) are rank-1
matmuls against per-sample statistics.
"""

import math
from contextlib import ExitStack

import ml_dtypes
import numpy as np

import concourse.bass as bass
import concourse.bacc as bacc_mod
import concourse.mybir as mybir
import concourse.tile as tile
from concourse.bass_utils import run_bass_kernel_spmd

D, H, HD, S, L, B = 768, 8, 96, 23, 50, 4096
NCORES = 8
BC = B // NCORES              # samples per core
ROWS = BC * S                 # 11776 rows per core
NB = 16                       # samples per block
NBLK = BC // NB               # 32 blocks
N = NB * S                    # 368 rows per block (div by 16 for XBAR)
GS = [(0, 128), (128, 256), (256, 368)]   # row-chunks within a block
NH = 9                        # 8 attention heads + 1 doc-mean "head"
NEG = -30000.0

F32 = mybir.dt.float32
BF16 = mybir.dt.bfloat16
BF = ml_dtypes.bfloat16


def _host_prep(cls_emb, missing_table, in_proj_w, in_proj_b,
               out_proj_w, out_proj_b, pred_w, pred_b, exist_mask):
    f32 = np.float32
    x = np.asarray(cls_emb, f32)
    mt = np.asarray(missing_table, f32)
    ipw = np.asarray(in_proj_w, f32)
    ipb = np.asarray(in_proj_b, f32)
    opw = np.asarray(out_proj_w, f32)
    opb = np.asarray(out_proj_b, f32)
    pw = np.asarray(pred_w, f32)
    pb = np.asarray(pred_b, f32)
    em = np.asarray(exist_mask)

    Wq, Wk, Wv = ipw[0:D], ipw[D:2 * D], ipw[2 * D:3 * D]
    bq, bk, bv = ipb[0:D], ipb[D:2 * D], ipb[2 * D:3 * D]
    scale = 1.0 / math.sqrt(HD)
    qm = mt @ Wq.T + bq
    qh = qm.reshape(S, H, HD)
    Wk3 = Wk.reshape(H, HD, D)
    Wv3 = Wv.reshape(H, HD, D)
    U = (np.einsum('hij,qhi->hqj', Wk3, qh) * scale).reshape(H * S, D)
    c0 = (np.einsum('qhi,hi->hq', qh, bk.reshape(H, HD)) * scale).reshape(H * S)
    W2 = pw @ opw
    Mcat = np.einsum('lhi,hid->hld', W2.reshape(L, H, HD), Wv3).reshape(H * L, D) / S
    vbs = ((W2 @ bv + pw @ opb) / S).astype(f32)
    wpts = ((pw @ mt.sum(0)) / S).astype(f32)

    UT = np.ascontiguousarray(U.T).astype(BF)                     # [768,184]
    MT = np.ascontiguousarray(
        np.concatenate([Mcat.T, (pw / S).T], axis=1)).astype(BF)  # [768,450]
    c0col = np.ascontiguousarray(c0.reshape(H * S, 1), f32)
    k1neg = np.full((1, H * S), NEG, f32).astype(BF)
    k1c = np.stack([vbs, wpts, pb]).astype(f32)                   # [3,50] -> flat rows
    k1c = np.ascontiguousarray(k1c.reshape(1, 3 * L))             # [1,150]

    # one-hot head reducer: rows 0..183 = (h,q) -> col h; row 184 = m -> col 8
    oh = np.zeros((H * S + 1, NH), f32)
    for i in range(H * S):
        oh[i, i // S] = 1.0
    oh[H * S, 8] = 1.0
    oh = oh.astype(BF)

    # block one-hot masks: maskm[r, h*NB + b] = 1 iff r//S == b
    maskm = np.zeros((N, NH * NB), f32)
    for r in range(N):
        b = r // S
        for h in range(NH):
            maskm[r, h * NB + b] = 1.0
    maskm = maskm.astype(BF)

    m = em.astype(f32)                                            # [B,S]
    hasany = (m.sum(1) > 0).astype(f32)
    u = (1.0 - m) * hasany[:, None]
    nupd = u.sum(1)

    xb = x.reshape(B * S, D).astype(BF)

    shards = []
    for c in range(NCORES):
        b0, b1 = c * BC, (c + 1) * BC
        xc = np.zeros((ROWS + 16, D), BF)
        xc[:ROWS] = xb[b0 * S:b1 * S]
        ms = m[b0:b1].reshape(-1)                                 # [ROWS]
        # per-block packed [m-row ; (1-m)-row]
        mr2 = np.stack([ms, 1.0 - ms], axis=0).reshape(2, NBLK, N)
        mr2 = np.ascontiguousarray(mr2.transpose(1, 0, 2)).reshape(1, 2 * ROWS)
        urep = np.zeros((H * S, BC), f32)
        for h in range(H):
            urep[h * S:(h + 1) * S, :] = u[b0:b1].T
        # packed [128, 2*BC]: first half rows 0..127, second half rows
        # 128..183 on partitions 0..55 (pad rows zeroed)
        urp = np.zeros((128, 2 * BC), f32)
        urp[:, 0:BC] = urep[0:128]
        urp[0:56, BC:2 * BC] = urep[128:184]
        sv = np.stack([nupd[b0:b1], 1.0 - hasany[b0:b1],
                       np.ones(BC, f32)]).astype(f32).reshape(1, 3 * BC)
        shards.append({
            "x": xc, "mrow": mr2.astype(BF),
            "ur": urp, "sv": np.ascontiguousarray(sv),
            "ut": UT, "mt": MT, "oh": oh, "maskm": maskm,
            "k1neg": k1neg, "c0col": c0col, "k1c": k1c,
        })
    return shards


def _build_program(ablate=5):
    nc = bacc_mod.Bacc("TRN2", target_bir_lowering=False, debug=False)
    x_d = nc.dram_tensor("x", [ROWS + 16, D], BF16, kind="ExternalInput").ap()
    mrow_d = nc.dram_tensor("mrow", [1, 2 * ROWS], BF16, kind="ExternalInput").ap()
    ur_d = nc.dram_tensor("ur", [128, 2 * BC], F32, kind="ExternalInput").ap()
    sv_d = nc.dram_tensor("sv", [1, 3 * BC], F32, kind="ExternalInput").ap()
    ut_d = nc.dram_tensor("ut", [D, H * S], BF16, kind="ExternalInput").ap()
    mt_d = nc.dram_tensor("mt", [D, NH * L], BF16, kind="ExternalInput").ap()
    oh_d = nc.dram_tensor("oh", [H * S + 1, NH], BF16, kind="ExternalInput").ap()
    mask_d = nc.dram_tensor("maskm", [N, NH * NB], BF16, kind="ExternalInput").ap()
    k1neg_d = nc.dram_tensor("k1neg", [1, H * S], BF16, kind="ExternalInput").ap()
    c0_d = nc.dram_tensor("c0col", [H * S, 1], F32, kind="ExternalInput").ap()
    k1c_d = nc.dram_tensor("k1c", [1, 3 * L], F32, kind="ExternalInput").ap()
    out_d = nc.dram_tensor("logitsT", [L, BC], F32, kind="ExternalOutput").ap()

    EXP = mybir.ActivationFunctionType.Exp

    def mm(out, lhsT, rhs, start, stop):
        nc.tensor.matmul(out, lhsT, rhs, start=start, stop=stop)

    with tile.TileContext(nc) as tc, ExitStack() as ctx:
        cpool = ctx.enter_context(tc.tile_pool(name="consts", bufs=1))
        natp = ctx.enter_context(tc.tile_pool(name="xnat", bufs=5))
        xtp = ctx.enter_context(tc.tile_pool(name="xts", bufs=5))
        ewp = ctx.enter_context(tc.tile_pool(name="ew", bufs=3))
        smp = ctx.enter_context(tc.tile_pool(name="small", bufs=3))
        abp = ctx.enter_context(tc.tile_pool(name="abm", bufs=3))
        xsp = ctx.enter_context(tc.tile_pool(name="xwsb", bufs=3))
        outp = ctx.enter_context(tc.tile_pool(name="outp", bufs=1))
        yp = ctx.enter_context(tc.tile_pool(name="ypsum", bufs=2, space="PSUM"))
        xwp = ctx.enter_context(tc.tile_pool(name="xwpsum", bufs=2, space="PSUM"))
        a9p = ctx.enter_context(tc.tile_pool(name="a9psum", bufs=2, space="PSUM"))

        def cload(name, shape, src, cdt=F32):
            t = cpool.tile(shape, cdt, tag=name, name=name)
            nc.sync.dma_start(t[:], src)
            return t

        ut_sb = [cload(f"ut{dc}", [128, H * S], ut_d[dc * 128:(dc + 1) * 128, :], BF16)
                 for dc in range(6)]
        mt_sb = [cload(f"mt{dc}", [128, NH * L], mt_d[dc * 128:(dc + 1) * 128, :], BF16)
                 for dc in range(6)]
        oh0 = cload("oh0", [128, NH], oh_d[0:128, :], BF16)
        oh1 = cload("oh1", [56, NH], oh_d[128:184, :], BF16)
        ohm = cload("ohm", [1, NH], oh_d[184:185, :], BF16)
        km = [cload(f"km{g}", [g1 - g0, NH * NB], mask_d[g0:g1, :], BF16)
              for g, (g0, g1) in enumerate(GS)]
        k1neg_sb = cload("k1neg", [1, H * S], k1neg_d[:, :], BF16)
        c00 = cload("c00", [128, 1], c0_d[0:128, :])
        c01 = cload("c01", [56, 1], c0_d[128:184, :])
        urp = cload("urp", [128, 2 * BC], ur_d[:, :])
        sv_sb = cload("sv", [1, 3 * BC], sv_d[:, :])
        k1c_sb = cload("k1c", [1, 3 * L], k1c_d[:, :])
        mrow_sb = cload("mrow", [1, 2 * ROWS], mrow_d[:, :], BF16)
        outT = outp.tile([L, BC], F32, tag="outT", name="outT")

        def mark(blk, phase):
            try:
                PHASES.append((blk, phase, list(nc.all_instructions())[-1].name))
            except Exception:
                pass

        # per-block tile handles, keyed by block index
        tiles = {}

        def emit_dma(b):
            # xts issues on ACT: its buffer-free wait (y-matmul reads, early
            # in the chain) never blocks the exp instructions behind it.
            # xnat issues on SP where its late buffer-free wait (xw reads)
            # blocks nothing else.
            xts = xtp.tile([128, 6, N], BF16, tag="xts", name=f"xts{b}")
            nc.sync.dma_start_transpose(xts[:], x_d[b * N:b * N + N, :])
            xnat = natp.tile([128, 3, D], BF16, tag="xnat", name=f"xnat{b}")
            nc.scalar.dma_start(
                xnat[:], x_d[b * N:b * N + 384, :].rearrange("(g p) d -> p g d", p=128))
            tiles[b] = {"xts": xts, "xnat": xnat}
            mark(b, 'dma')

        def emit_scores(b):
            r0 = b * N
            xts = tiles[b]["xts"]
            mrn = mrow_sb[:, 2 * r0 + N:2 * r0 + 2 * N]
            y0 = yp.tile([128, N], F32, tag="y0", name=f"y0_{b}")
            y1 = yp.tile([56, N], F32, tag="y1", name=f"y1_{b}")
            for dc in range(6):
                mm(y0[:], ut_sb[dc][:, 0:128], xts[:, dc, :],
                   start=(dc == 0), stop=False)
                mm(y1[:], ut_sb[dc][:, 128:184], xts[:, dc, :],
                   start=(dc == 0), stop=False)
            mm(y0[:], k1neg_sb[:, 0:128], mrn, start=False, stop=True)
            mm(y1[:], k1neg_sb[:, 128:184], mrn, start=False, stop=True)
            tiles[b].update(y0=y0, y1=y1)
            mark(b, 'y')

        def emit_softmax(b):
            b0 = b * NB
            y0, y1 = tiles[b]["y0"], tiles[b]["y1"]
            # packed halves: [:, 0:N] = rows 0..127, [0:56, N:2N] = rows
            # 128..183. Pad rows (56:128 of the second half) hold garbage
            # that is masked by urp's zero pad and never read downstream.
            ew = ewp.tile([128, 2 * N], BF16, tag="ew", name=f"ew{b}")
            nc.scalar.activation(ew[:, 0:N], y0[:], EXP, bias=c00[:])
            nc.scalar.activation(ew[0:56, N:2 * N], y1[:], EXP, bias=c01[:])
            den = smp.tile([128, 2 * NB], F32, tag="den", name=f"den{b}")
            nc.vector.tensor_reduce(
                den[:].rearrange("p (t b) -> p t b", t=2),
                ew[:].rearrange("p (t b k) -> p t b k", t=2, k=S),
                axis=mybir.AxisListType.X, op=mybir.AluOpType.add)
            nc.vector.tensor_scalar_add(den[:], den[:], 1e-30)
            up = smp.tile([128, 2 * NB], F32, tag="up", name=f"up{b}")
            nc.vector.reciprocal(up[:], den[:])
            nc.gpsimd.tensor_mul(
                up[:].rearrange("p (t b) -> p t b", t=2),
                up[:].rearrange("p (t b) -> p t b", t=2),
                urp[:].rearrange("p (t c) -> p t c", t=2)[:, :, b0:b0 + NB])
            w = ewp.tile([128, 2 * N], BF16, tag="w", name=f"w{b}")
            bc = (up[:].rearrange("p (t b o) -> p t b o", t=2, o=1)
                  .broadcast_to([128, 2, NB, S]))
            nc.gpsimd.tensor_mul(
                w[:].rearrange("p (t b k) -> p t b k", t=2, k=S),
                ew[:].rearrange("p (t b k) -> p t b k", t=2, k=S), bc)
            tiles[b].update(w=w)
            mark(b, 'w')

        def emit_value(b):
            r0 = b * N
            b0 = b * NB
            w = tiles[b]["w"]
            xnat = tiles[b]["xnat"]
            mrp = mrow_sb[:, 2 * r0:2 * r0 + N]
            # per-(row, head) weight sums, 9th col = m (doc-mean).
            # All matmuls first, then all ab reads — interleaving would
            # create tile-granular WAR stalls between PE and DVE.
            # ct (logits accum) shares the bank as columns 27:43.
            act = a9p.tile([128, 3 * NH + NB], F32, tag="a9", name=f"a9_{b}")
            a9 = act[:, 0:3 * NH]
            ct = act[0:L, 3 * NH:3 * NH + NB]
            for g, (g0, g1) in enumerate(GS):
                sl = a9[0:g1 - g0, g * NH:(g + 1) * NH]
                mm(sl, w[:, g0:g1], oh0[:], start=True, stop=False)
                mm(sl, w[0:56, N + g0:N + g1], oh1[:], start=False, stop=False)
                mm(sl, mrp[:, g0:g1], ohm[:], start=False, stop=True)
            ab = []
            for g, (g0, g1) in enumerate(GS):
                pg = g1 - g0
                abg = abp.tile([pg, NH, NB], BF16, tag=f"ab{g}", name=f"ab{b}_{g}")
                nc.vector.tensor_mul(
                    abg[:],
                    a9[0:pg, g * NH:(g + 1) * NH]
                    .rearrange("p (j o) -> p j o", o=1).broadcast_to([pg, NH, NB]),
                    km[g][:].rearrange("p (j b) -> p j b", b=NB))
                ab.append(abg)
            mark(b, 'ab')
            if ablate < 4:
                del tiles[b]
                return

            # xw[d, (h,b)] = sum_r x[r,d] * Abm[r,(h,b)] — two half-passes
            # (dc 0-2, dc 3-5) over one double-buffered PSUM tag. Sample
            # spans per row-chunk: g0 rows cover b 0..5, g1 b 5..11, g2 b
            # 11..15. g0 writes full width (start resets the bank region);
            # g1/g2 accumulate only their local columns.
            SPAN = [(0, NB), (5, 12), (11, NB)]
            xss = []
            for half in range(2):
                xwh = xwp.tile([128, 3, NH, NB], F32, tag="xwH",
                               name=f"xw{b}_{half}")
                for dci in range(3):
                    for g, (g0, g1) in enumerate(GS):
                        bo0, bo1 = SPAN[g]
                        o = xwh[:, dci, :, bo0:bo1]
                        dc = half * 3 + dci
                        mm(o, xnat[0:g1 - g0, g, dc * 128:(dc + 1) * 128],
                           ab[g][:, :, bo0:bo1], start=(g == 0), stop=(g == 2))
                xsh = xsp.tile([128, 3, NH, NB], BF16, tag=f"xs{half}",
                               name=f"xs{b}_{half}")
                nc.vector.tensor_copy(xsh[:], xwh[:])
                xss.append(xsh)
            xsA, xsB = xss
            mark(b, 'xs')
            if ablate < 5:
                del tiles[b]
                return

            # logits: 54 head matmuls + 3 rank-1 constant terms
            first = True
            for dc in range(6):
                xsb = xsA if dc < 3 else xsB
                for h in range(NH):
                    mm(ct[:], mt_sb[dc][:, h * L:(h + 1) * L], xsb[:, dc % 3, h, :],
                       start=first, stop=False)
                    first = False
            for j in range(3):
                mm(ct[:], k1c_sb[:, j * L:(j + 1) * L],
                   sv_sb[:, j * BC + b0:j * BC + b0 + NB],
                   start=False, stop=(j == 2))
            nc.scalar.copy(outT[:, b0:b0 + NB], ct[:])
            del tiles[b]
            mark(b, 'ct')

        # software-pipelined emission: the scheduler's priority follows
        # emission order, so loads run 3 blocks ahead, scores 2, softmax 1
        # ahead of the current block's value-path consumption
        emit_dma(0)
        emit_dma(1)
        emit_dma(2)
        emit_scores(0)
        emit_scores(1)
        if ablate >= 2:
            emit_softmax(0)
        for blk in range(NBLK):
            if blk + 3 < NBLK:
                emit_dma(blk + 3)
            if blk + 2 < NBLK:
                emit_scores(blk + 2)
            if ablate >= 2 and blk + 1 < NBLK:
                emit_softmax(blk + 1)
            if ablate >= 3:
                emit_value(blk)

        nc.sync.dma_start(out_d[:, :], outT[:])
    nc.compile()
    return nc


PHASES = []


_CACHED = {}


def _get_program():
    if "nc" not in _CACHED:
        _CACHED["nc"] = _build_program()
    return _CACHED["nc"]


def _run(inputs, trace=False):
    shards = _host_prep(**inputs)
    nc = _get_program()
    res = run_bass_kernel_spmd(nc, shards, list(range(NCORES)), trace=trace)
    outs = [res.results[i]["logitsT"] for i in range(NCORES)]
    full = np.concatenate(outs, axis=1).T.astype(np.float32)
    return full, res


def kernel(**inputs):
    out, _ = _run(inputs, trace=False)
    return out


def run_traced(inputs):
    return _run(inputs, trace=True)
